# revision 10
# baseline (speedup 1.0000x reference)
"""CTC greedy decoder on 8 Trainium2 NeuronCores.

Data-parallel over batch: core i decodes sequences [4*i, 4*i+4).
Per core:
  phase 1: DMA x chunks (128t x 4b x 512v), DVE max/max_index -> argmax
           indices as (128t x 1) columns.
  phase 2: PE transposes (identity matmul) the index columns into a
           (4b x 2048t) PSUM tile.
  phase 3: DVE builds keep mask + cumsum (tensor_tensor_scan), GPSIMD
           local_scatter stream-compacts kept tokens (two 1024-slot
           halves); empty slots come out as -1 via the +1/-1 trick.
"""

import numpy as np

SEQ, BATCH, VOCAB = 2048, 32, 512
N_CORES = 8
BPC = BATCH // N_CORES  # sequences per core
NCHUNK = SEQ // 128  # time chunks of 128
P16 = 16  # gpsimd channel granularity

_cache = {}


def _build(blank: float):
    import concourse.tile as tile
    from concourse import bacc, mybir
    from concourse.masks import make_identity

    f32 = mybir.dt.float32
    i32 = mybir.dt.int32
    i16 = mybir.dt.int16
    u32 = mybir.dt.uint32
    Alu = mybir.AluOpType

    nc = bacc.Bacc("TRN2", target_bir_lowering=False, debug=False)
    x_d = nc.dram_tensor("x", (SEQ, BPC, VOCAB), f32, kind="ExternalInput")
    len_d = nc.dram_tensor("lengths", (BPC, 1), i32, kind="ExternalInput")
    tok_d = nc.dram_tensor("tokens", (BPC, SEQ), i32, kind="ExternalOutput")
    olen_d = nc.dram_tensor("out_lengths", (BPC, 1), i32, kind="ExternalOutput")

    with tile.TileContext(nc) as tc:
        with (
            tc.tile_pool(name="consts", bufs=1) as consts,
            tc.tile_pool(name="xin", bufs=3) as xin,
            tc.tile_pool(name="am", bufs=1) as am,
            tc.tile_pool(name="mx", bufs=2) as mxp,
            tc.tile_pool(name="psum", bufs=1, space="PSUM") as psum,
            tc.tile_pool(name="seq", bufs=1) as seq,
        ):
            ident = consts.tile([128, 128], f32)
            make_identity(nc, ident[:])

            AM = am.tile([128, NCHUNK * BPC, 8], u32)
            AMf = am.tile([128, NCHUNK * BPC], f32)
            PT = psum.tile([BPC, SEQ], f32)

            # phase 1+2: argmax over vocab, transpose to (b, t)
            for c in range(NCHUNK):
                X = xin.tile([128, BPC, VOCAB], f32, tag="X")
                nc.sync.dma_start(X[:], x_d[c * 128 : (c + 1) * 128, :, :])
                for b in range(BPC):
                    j = c * BPC + b
                    mx8 = mxp.tile([128, 8], f32, tag="mx")
                    nc.vector.max(out=mx8[:], in_=X[:, b, :])
                    nc.vector.max_index(
                        out=AM[:, j, :], in_max=mx8[:], in_values=X[:, b, :]
                    )
                nc.vector.tensor_copy(
                    AMf[:, c * BPC : (c + 1) * BPC], AM[:, c * BPC : (c + 1) * BPC, 0]
                )
                nc.tensor.transpose(
                    PT[:, c * 128 : (c + 1) * 128],
                    AMf[:, c * BPC : (c + 1) * BPC],
                    ident[:],
                )

            # phase 3: sequence dedup in (b, t) layout, rows 4..15 inert
            def s(tag, dtype=f32, shape=None):
                return seq.tile(shape or [P16, SEQ], dtype, tag=tag, name=tag)

            mlf = s("S1")
            nc.vector.memset(mlf[:], 0.0)
            nc.vector.tensor_copy(mlf[0:BPC, :], PT[:])

            lens_i = seq.tile([P16, 1], i32, tag="LENS")
            nc.vector.memset(lens_i[:], 0)
            nc.sync.dma_start(lens_i[0:BPC, :], len_d[:])
            lens_f = seq.tile([P16, 1], f32, tag="LENSF")
            nc.vector.tensor_copy(lens_f[:], lens_i[:])

            iota_i = s("S8", i32)
            nc.gpsimd.iota(iota_i[:], pattern=[[1, SEQ]], base=0, channel_multiplier=0)
            iota_f = s("S9")
            nc.vector.tensor_copy(iota_f[:], iota_i[:])
            valid = s("S4")
            nc.vector.tensor_scalar(
                out=valid[:], in0=iota_f[:], scalar1=lens_f[:], scalar2=None,
                op0=Alu.is_lt,
            )

            prevf = s("S2")
            nc.vector.memset(prevf[:, 0:1], -1.0)
            nc.vector.tensor_copy(prevf[:, 1:SEQ], mlf[:, 0 : SEQ - 1])

            chg = s("S3")
            nc.vector.tensor_tensor(out=chg[:], in0=mlf[:], in1=prevf[:], op=Alu.not_equal)
            k0 = s("S5")
            nc.vector.scalar_tensor_tensor(
                out=k0[:], in0=prevf[:], scalar=blank, in1=chg[:],
                op0=Alu.is_equal, op1=Alu.logical_or,
            )
            nbv = s("S6")
            nc.vector.scalar_tensor_tensor(
                out=nbv[:], in0=mlf[:], scalar=blank, in1=valid[:],
                op0=Alu.not_equal, op1=Alu.mult,
            )
            keep = s("S7")
            nc.vector.tensor_tensor(out=keep[:], in0=k0[:], in1=nbv[:], op=Alu.mult)

            zeros = s("S8")  # reuses iota_i slot
            nc.vector.memset(zeros[:], 0.0)
            pos = s("S2")  # reuses prevf slot
            nc.vector.tensor_tensor_scan(
                out=pos[:], data0=keep[:], data1=zeros[:], initial=0.0,
                op0=Alu.add, op1=Alu.add,
            )

            t0 = s("S3")  # reuses chg slot
            nc.vector.tensor_tensor(out=t0[:], in0=pos[:], in1=keep[:], op=Alu.mult)
            idxf = s("S6")  # reuses nbv slot
            nc.vector.tensor_scalar(
                out=idxf[:], in0=t0[:], scalar1=1.0, scalar2=None, op0=Alu.subtract
            )
            hi = s("S5")  # reuses k0 slot
            nc.vector.tensor_scalar(
                out=hi[:], in0=idxf[:], scalar1=float(SEQ // 2), scalar2=None,
                op0=Alu.is_ge,
            )
            idx1f = s("S3")  # reuses t0 slot
            nc.vector.scalar_tensor_tensor(
                out=idx1f[:], in0=hi[:], scalar=float(-SEQ), in1=idxf[:],
                op0=Alu.mult, op1=Alu.add,
            )
            idx2f = s("S4")  # reuses valid slot
            nc.vector.tensor_scalar(
                out=idx2f[:], in0=idxf[:], scalar1=float(SEQ // 2), scalar2=None,
                op0=Alu.subtract,
            )

            idx1_i = s("I1", i16)
            nc.vector.tensor_copy(idx1_i[:], idx1f[:])
            idx2_i = s("I2", i16)
            nc.vector.tensor_copy(idx2_i[:], idx2f[:])
            data_i = s("D", i16)
            nc.vector.tensor_scalar(
                out=data_i[:], in0=mlf[:], scalar1=1.0, scalar2=None, op0=Alu.add
            )

            dst1 = seq.tile([P16, SEQ // 2], i16, tag="DST1")
            dst2 = seq.tile([P16, SEQ // 2], i16, tag="DST2")
            nc.gpsimd.local_scatter(
                out_ap=dst1[:], data_ap=data_i[:], idxs_ap=idx1_i[:],
                channels=P16, num_elems=SEQ // 2, num_idxs=SEQ,
            )
            nc.gpsimd.local_scatter(
                out_ap=dst2[:], data_ap=data_i[:], idxs_ap=idx2_i[:],
                channels=P16, num_elems=SEQ // 2, num_idxs=SEQ,
            )

            tok = seq.tile([BPC, SEQ], i32, tag="TOK")
            nc.vector.tensor_scalar(
                out=tok[:, 0 : SEQ // 2], in0=dst1[0:BPC, :], scalar1=1,
                scalar2=None, op0=Alu.subtract,
            )
            nc.vector.tensor_scalar(
                out=tok[:, SEQ // 2 : SEQ], in0=dst2[0:BPC, :], scalar1=1,
                scalar2=None, op0=Alu.subtract,
            )
            olen = seq.tile([BPC, 1], i32, tag="OLEN")
            nc.vector.tensor_copy(olen[:], pos[0:BPC, SEQ - 1 : SEQ])

            nc.sync.dma_start(tok_d[:], tok[:])
            nc.sync.dma_start(olen_d[:], olen[:])

    nc.compile()
    return nc


def get_program(blank: float):
    key = float(blank)
    if key not in _cache:
        _cache[key] = _build(key)
    return _cache[key]


def make_in_maps(x: np.ndarray, lengths: np.ndarray):
    x = np.asarray(x, dtype=np.float32)
    lengths = np.asarray(lengths, dtype=np.int32)
    in_maps = []
    for i in range(N_CORES):
        sl = slice(i * BPC, (i + 1) * BPC)
        in_maps.append(
            {
                "x": np.ascontiguousarray(x[:, sl, :]),
                "lengths": np.ascontiguousarray(lengths[sl].reshape(BPC, 1)),
            }
        )
    return in_maps


def assemble(results):
    tokens = np.concatenate([r["tokens"] for r in results], axis=0).astype(np.int32)
    out_lengths = (
        np.concatenate([r["out_lengths"] for r in results], axis=0)
        .reshape(BATCH)
        .astype(np.int32)
    )
    return tokens, out_lengths


def kernel(x, lengths, blank_index):
    from concourse.bass_utils import run_bass_kernel_spmd

    nc = get_program(float(np.asarray(blank_index)))
    in_maps = make_in_maps(x, lengths)
    res = run_bass_kernel_spmd(nc, in_maps, list(range(N_CORES)))
    return assemble(res.results)


# revision 14
# speedup vs baseline: 984.6412x; 984.6412x over previous
"""CTC greedy decoder on 8 Trainium2 NeuronCores.

Data-parallel over batch: core i decodes sequences [4*i, 4*i+4).
Per core:
  phase 1: DMA x chunks (128t x 4b x 512v), DVE max/max_index -> argmax
           indices as (128t x 1) columns.
  phase 2: PE transposes (identity matmul) the index columns into a
           (4b x 2048t) PSUM tile.
  phase 3: DVE builds keep mask + cumsum (tensor_tensor_scan), GPSIMD
           local_scatter stream-compacts kept tokens (two 1024-slot
           halves); empty slots come out as -1 via the +1/-1 trick.

`repeat` builds R back-to-back iterations of the whole pipeline inside
one NEFF — used by test.py to measure steady-state per-iteration time
(the single-shot dispatch overhead through the axon relay is ~88 ms,
so per-exec time is recovered from the wall-clock slope vs R).
"""

import numpy as np

SEQ, BATCH, VOCAB = 2048, 32, 512
N_CORES = 8
BPC = BATCH // N_CORES  # sequences per core
NCHUNK = SEQ // 128  # time chunks of 128
P16 = 16  # gpsimd channel granularity

_cache = {}


def _build(blank: float, repeat: int = 1):
    import concourse.tile as tile
    from concourse import bacc, mybir
    from concourse.masks import make_identity

    f32 = mybir.dt.float32
    i32 = mybir.dt.int32
    i16 = mybir.dt.int16
    u32 = mybir.dt.uint32
    Alu = mybir.AluOpType

    nc = bacc.Bacc("TRN2", target_bir_lowering=False, debug=False)
    x_d = nc.dram_tensor("x", (SEQ, BPC, VOCAB), f32, kind="ExternalInput")
    len_d = nc.dram_tensor("lengths", (BPC, 1), i32, kind="ExternalInput")
    tok_d = nc.dram_tensor("tokens", (BPC, SEQ), i32, kind="ExternalOutput")
    olen_d = nc.dram_tensor("out_lengths", (BPC, 1), i32, kind="ExternalOutput")

    with tile.TileContext(nc) as tc:
        with (
            tc.tile_pool(name="consts", bufs=1) as consts,
            tc.tile_pool(name="xin", bufs=3) as xin,
            tc.tile_pool(name="am", bufs=2) as am,
            tc.tile_pool(name="mx", bufs=2) as mxp,
            tc.tile_pool(name="psum", bufs=2, space="PSUM") as psum,
            tc.tile_pool(name="seq", bufs=1) as seq,
        ):
            ident = consts.tile([128, 128], f32)
            make_identity(nc, ident[:])

            for _rep in range(repeat):
                one_iter(nc, tc, mybir, Alu, blank, x_d, len_d, tok_d, olen_d,
                         ident, xin, am, mxp, psum, seq)

    nc.compile()
    return nc


def one_iter(nc, tc, mybir, Alu, blank, x_d, len_d, tok_d, olen_d,
             ident, xin, am, mxp, psum, seq):
    f32 = mybir.dt.float32
    i32 = mybir.dt.int32
    i16 = mybir.dt.int16
    u32 = mybir.dt.uint32

    AM = am.tile([128, NCHUNK * BPC, 8], u32, tag="AM", name="AM")
    AMf = am.tile([128, NCHUNK * BPC], f32, tag="AMf", name="AMf")
    PT = psum.tile([BPC, SEQ], f32, tag="PT", name="PT")

    # phase 1+2: argmax over vocab, transpose to (b, t)
    for c in range(NCHUNK):
        X = xin.tile([128, BPC, VOCAB], f32, tag="X", name="X")
        nc.sync.dma_start(X[:], x_d[c * 128 : (c + 1) * 128, :, :])
        for b in range(BPC):
            j = c * BPC + b
            mx8 = mxp.tile([128, 8], f32, tag="mx", name="mx")
            nc.vector.max(out=mx8[:], in_=X[:, b, :])
            nc.vector.max_index(
                out=AM[:, j, :], in_max=mx8[:], in_values=X[:, b, :]
            )
        nc.vector.tensor_copy(
            AMf[:, c * BPC : (c + 1) * BPC], AM[:, c * BPC : (c + 1) * BPC, 0]
        )
        nc.tensor.transpose(
            PT[:, c * 128 : (c + 1) * 128],
            AMf[:, c * BPC : (c + 1) * BPC],
            ident[:],
        )

    # phase 3: sequence dedup in (b, t) layout, rows 4..15 inert
    def s(tag, dtype=f32, shape=None):
        return seq.tile(shape or [P16, SEQ], dtype, tag=tag, name=tag)

    mlf = s("S1")
    nc.vector.memset(mlf[:], 0.0)
    nc.vector.tensor_copy(mlf[0:BPC, :], PT[:])

    lens_i = seq.tile([P16, 1], i32, tag="LENS", name="LENS")
    nc.vector.memset(lens_i[:], 0)
    nc.sync.dma_start(lens_i[0:BPC, :], len_d[:])
    lens_f = seq.tile([P16, 1], f32, tag="LENSF", name="LENSF")
    nc.vector.tensor_copy(lens_f[:], lens_i[:])

    iota_i = s("S8", i32)
    nc.gpsimd.iota(iota_i[:], pattern=[[1, SEQ]], base=0, channel_multiplier=0)
    iota_f = s("S9")
    nc.vector.tensor_copy(iota_f[:], iota_i[:])
    valid = s("S4")
    nc.vector.tensor_scalar(
        out=valid[:], in0=iota_f[:], scalar1=lens_f[:], scalar2=None,
        op0=Alu.is_lt,
    )

    prevf = s("S2")
    nc.vector.memset(prevf[:, 0:1], -1.0)
    nc.vector.tensor_copy(prevf[:, 1:SEQ], mlf[:, 0 : SEQ - 1])

    chg = s("S3")
    nc.vector.tensor_tensor(out=chg[:], in0=mlf[:], in1=prevf[:], op=Alu.not_equal)
    k0 = s("S5")
    nc.vector.scalar_tensor_tensor(
        out=k0[:], in0=prevf[:], scalar=blank, in1=chg[:],
        op0=Alu.is_equal, op1=Alu.logical_or,
    )
    nbv = s("S6")
    nc.vector.scalar_tensor_tensor(
        out=nbv[:], in0=mlf[:], scalar=blank, in1=valid[:],
        op0=Alu.not_equal, op1=Alu.mult,
    )
    keep = s("S7")
    nc.vector.tensor_tensor(out=keep[:], in0=k0[:], in1=nbv[:], op=Alu.mult)

    zeros = s("S8")  # reuses iota_i slot
    nc.vector.memset(zeros[:], 0.0)
    pos = s("S2")  # reuses prevf slot
    nc.vector.tensor_tensor_scan(
        out=pos[:], data0=keep[:], data1=zeros[:], initial=0.0,
        op0=Alu.add, op1=Alu.add,
    )

    t0 = s("S3")  # reuses chg slot
    nc.vector.tensor_tensor(out=t0[:], in0=pos[:], in1=keep[:], op=Alu.mult)
    idxf = s("S6")  # reuses nbv slot
    nc.vector.tensor_scalar(
        out=idxf[:], in0=t0[:], scalar1=1.0, scalar2=None, op0=Alu.subtract
    )
    hi = s("S5")  # reuses k0 slot
    nc.vector.tensor_scalar(
        out=hi[:], in0=idxf[:], scalar1=float(SEQ // 2), scalar2=None,
        op0=Alu.is_ge,
    )
    idx1f = s("S3")  # reuses t0 slot
    nc.vector.scalar_tensor_tensor(
        out=idx1f[:], in0=hi[:], scalar=float(-SEQ), in1=idxf[:],
        op0=Alu.mult, op1=Alu.add,
    )
    idx2f = s("S4")  # reuses valid slot
    nc.vector.tensor_scalar(
        out=idx2f[:], in0=idxf[:], scalar1=float(SEQ // 2), scalar2=None,
        op0=Alu.subtract,
    )

    idx1_i = s("I1", i16)
    nc.vector.tensor_copy(idx1_i[:], idx1f[:])
    idx2_i = s("I2", i16)
    nc.vector.tensor_copy(idx2_i[:], idx2f[:])
    data_i = s("D", i16)
    nc.vector.tensor_scalar(
        out=data_i[:], in0=mlf[:], scalar1=1.0, scalar2=None, op0=Alu.add
    )

    dst1 = seq.tile([P16, SEQ // 2], i16, tag="DST1", name="DST1")
    dst2 = seq.tile([P16, SEQ // 2], i16, tag="DST2", name="DST2")
    nc.gpsimd.local_scatter(
        out_ap=dst1[:], data_ap=data_i[:], idxs_ap=idx1_i[:],
        channels=P16, num_elems=SEQ // 2, num_idxs=SEQ,
    )
    nc.gpsimd.local_scatter(
        out_ap=dst2[:], data_ap=data_i[:], idxs_ap=idx2_i[:],
        channels=P16, num_elems=SEQ // 2, num_idxs=SEQ,
    )

    tok = seq.tile([BPC, SEQ], i32, tag="TOK", name="TOK")
    nc.vector.tensor_scalar(
        out=tok[:, 0 : SEQ // 2], in0=dst1[0:BPC, :], scalar1=1,
        scalar2=None, op0=Alu.subtract,
    )
    nc.vector.tensor_scalar(
        out=tok[:, SEQ // 2 : SEQ], in0=dst2[0:BPC, :], scalar1=1,
        scalar2=None, op0=Alu.subtract,
    )
    olen = seq.tile([BPC, 1], i32, tag="OLEN", name="OLEN")
    nc.vector.tensor_copy(olen[:], pos[0:BPC, SEQ - 1 : SEQ])

    nc.sync.dma_start(tok_d[:], tok[:])
    nc.sync.dma_start(olen_d[:], olen[:])


def get_program(blank: float, repeat: int = 1):
    key = (float(blank), int(repeat))
    if key not in _cache:
        _cache[key] = _build(*key)
    return _cache[key]


def make_in_maps(x: np.ndarray, lengths: np.ndarray):
    x = np.asarray(x, dtype=np.float32)
    lengths = np.asarray(lengths, dtype=np.int32)
    in_maps = []
    for i in range(N_CORES):
        sl = slice(i * BPC, (i + 1) * BPC)
        in_maps.append(
            {
                "x": np.ascontiguousarray(x[:, sl, :]),
                "lengths": np.ascontiguousarray(lengths[sl].reshape(BPC, 1)),
            }
        )
    return in_maps


def assemble(results):
    tokens = np.concatenate([r["tokens"] for r in results], axis=0).astype(np.int32)
    out_lengths = (
        np.concatenate([r["out_lengths"] for r in results], axis=0)
        .reshape(BATCH)
        .astype(np.int32)
    )
    return tokens, out_lengths


def kernel(x, lengths, blank_index):
    from concourse.bass_utils import run_bass_kernel_spmd

    nc = get_program(float(np.asarray(blank_index)))
    in_maps = make_in_maps(x, lengths)
    res = run_bass_kernel_spmd(nc, in_maps, list(range(N_CORES)))
    return assemble(res.results)
